# revision 32
# baseline (speedup 1.0000x reference)
"""Trainium2 Bass kernel for a causal multi-head attention layer.

Model: b=2, s=2048, d_model=1024, 16 heads, head_dim=64, pad-index 0.
Sharding over 8 NeuronCores: each core owns 2 heads (128 of the 1024
attention dims) for both batches (head/tensor parallel).  After attention,
an AllToAll redistributes the per-head outputs so each core holds all 1024
attention dims for 1/8 of the sequence positions, where it runs the output
projection locally.  Output rows per core: 256 rows of each batch.

Attention is computed in 512-query stripes: per stripe, scores for both
heads run as concurrent row-group matmuls (h0 in PE rows 0-63, h1 in
64-127), one exp covers both heads, and the PV matmul keeps V stationary
(with a ones column for the softmax denominator) so the output lands
directly in [dims, queries] layout for the AllToAll.
"""

import threading

import numpy as np

B, S, D = 2, 2048, 1024
H, HD = 16, 64
NCORES = 8
LD = D // NCORES          # 128 local attention dims (2 heads)
R = B * S                 # 4096 flattened rows
RC = R // NCORES          # 512 output rows per core
RB = S // NCORES          # 256 rows per batch per core
NKT = S // 128            # 16 key tiles per batch
NCH = D // 128            # 8 contraction chunks of d_model
NST = S // 512            # 4 query stripes per batch

_cache = {}
_lock = threading.Lock()


def _stripe_layout():
    """Per stripe c: list of (kt, width, q_start, offset-in-block), block len."""
    layout = []
    for c in range(NST):
        entries = []
        off = 0
        for kt in range(4 * c + 4):
            qs = max(512 * c, kt * 128)
            w = 512 * (c + 1) - qs
            entries.append((kt, w, qs, off))
            off += w
        layout.append((entries, off))
    return layout


def _build_nc():
    import concourse.mybir as mybir
    import concourse.tile as tile
    from concourse import bacc
    from contextlib import ExitStack

    f32 = mybir.dt.float32
    bf16 = mybir.dt.bfloat16
    i32 = mybir.dt.int32
    AF = mybir.ActivationFunctionType
    ALU = mybir.AluOpType

    nc = bacc.Bacc(None, target_bir_lowering=False, num_devices=NCORES)

    xT = nc.declare_dram_parameter("xT", [D, R], bf16, isOutput=False)
    wqT = nc.declare_dram_parameter("wqT", [D, LD], bf16, isOutput=False)
    wkT = nc.declare_dram_parameter("wkT", [D, LD], bf16, isOutput=False)
    wvT = nc.declare_dram_parameter("wvT", [D, LD], bf16, isOutput=False)
    woT = nc.declare_dram_parameter("woT", [D, D], bf16, isOutput=False)
    bq = nc.declare_dram_parameter("bq", [LD], f32, isOutput=False)
    bk = nc.declare_dram_parameter("bk", [LD], f32, isOutput=False)
    bv = nc.declare_dram_parameter("bv", [LD], f32, isOutput=False)
    bo = nc.declare_dram_parameter("bo", [D], f32, isOutput=False)
    ids = nc.declare_dram_parameter("ids", [128, B * NKT], i32, isOutput=False)
    out = nc.declare_dram_parameter("out", [RC, D], f32, isOutput=True)

    layout = _stripe_layout()

    with ExitStack() as ctx:
        tc = ctx.enter_context(tile.TileContext(nc))
        const = ctx.enter_context(tc.tile_pool(name="const", bufs=1))
        xcp = ctx.enter_context(tc.tile_pool(name="xcp", bufs=1))
        qkp = ctx.enter_context(tc.tile_pool(name="qkp", bufs=2))
        estp = ctx.enter_context(tc.tile_pool(name="estp", bufs=1))
        stg = ctx.enter_context(tc.tile_pool(name="stg", bufs=2))
        work = ctx.enter_context(tc.tile_pool(name="work", bufs=2))
        recp = ctx.enter_context(tc.tile_pool(name="recp", bufs=1))
        spool = ctx.enter_context(tc.tile_pool(name="spool", bufs=2, space="PSUM"))
        pvpool = ctx.enter_context(tc.tile_pool(name="pvpool", bufs=2, space="PSUM"))
        dpool = ctx.enter_context(tc.tile_pool(name="dram", bufs=2, space="DRAM"))

        # ---- constants (small weights first so compute can start early) ----
        wqT_sb = const.tile([128, NCH, LD], bf16)
        nc.sync.dma_start(wqT_sb, wqT.ap().rearrange("(c p) d -> p c d", p=128))
        wkT_sb = const.tile([128, NCH, LD], bf16)
        nc.sync.dma_start(wkT_sb, wkT.ap().rearrange("(c p) d -> p c d", p=128))
        wvT_sb = const.tile([128, NCH, LD], bf16)
        nc.sync.dma_start(wvT_sb, wvT.ap().rearrange("(c p) d -> p c d", p=128))

        bq_col = const.tile([128, 1], f32)
        nc.sync.dma_start(bq_col, bq.ap().rearrange("(p o) -> p o", o=1))
        bk_col = const.tile([128, 1], f32)
        nc.sync.dma_start(bk_col, bk.ap().rearrange("(p o) -> p o", o=1))
        bv_bc = const.tile([128, LD], f32)
        nc.sync.dma_start(bv_bc, bv.ap().partition_broadcast(128))

        ids_sb = const.tile([128, B * NKT], i32)
        nc.sync.dma_start(ids_sb, ids.ap())
        ones64 = const.tile([1, 64], bf16)
        nc.vector.memset(ones64, 1.0)

        # x^T for batch 0, chunked by (row-block, dim-chunk) so projection
        # matmuls start as soon as the first row block lands
        xTr = xT.ap().rearrange("(c p) r -> c p r", p=128)
        xc = [xcp.tile([128, S], bf16, name=f"xc{c}", tag=f"xc{c}")
              for c in range(NCH)]
        for rb in range(4):
            rsl = slice(rb * 512, (rb + 1) * 512)
            for c in range(NCH):
                nc.sync.dma_start(xc[c][:, rsl], xTr[c][:, rsl])

        woT_sb = const.tile([128, NCH, D], bf16)
        nc.sync.dma_start(woT_sb, woT.ap().rearrange("(c p) n -> p c n", p=128))
        bo_bc = const.tile([128, D], f32)
        nc.sync.dma_start(bo_bc, bo.ap().partition_broadcast(128))

        padf = const.tile([128, B * NKT], f32)
        nc.vector.tensor_copy(padf, ids_sb)
        nc.vector.tensor_scalar_min(padf, padf, 1.0)

        # diagmask2[x, h, y] = 1 if y >= x else 0 (keys on partitions)
        diagmask = const.tile([128, 128], bf16)
        nc.gpsimd.memset(diagmask, 1.0)
        nc.gpsimd.affine_select(
            out=diagmask, in_=diagmask, compare_op=ALU.is_ge, fill=0.0,
            base=0, pattern=[[1, 128]], channel_multiplier=-1,
        )
        diagmask2 = const.tile([128, 2, 128], bf16)
        nc.vector.tensor_copy(diagmask2[:, 0, :], diagmask)
        nc.vector.tensor_copy(diagmask2[:, 1, :], diagmask)

        a2a_outs = []
        for b in range(B):
            # ---- Q/K projections: [dims, rows], both heads stacked ----
            qt_sb = qkp.tile([128, S], bf16, name=f"qt{b}", tag="qt")
            kt_sb = qkp.tile([128, S], bf16, name=f"kt{b}", tag="kt")
            for rb in range(4):
                rsl = slice(rb * 512, (rb + 1) * 512)
                pqt = spool.tile([128, 512], f32, name="pqt", tag="ps")
                pkt = spool.tile([128, 512], f32, name="pkt", tag="ps")
                for c in range(NCH):
                    st = c == 0
                    sp = c == NCH - 1
                    rhs = xc[c][:, rsl]
                    nc.tensor.matmul(pqt, wqT_sb[:, c, :], rhs, start=st, stop=sp)
                    nc.tensor.matmul(pkt, wkT_sb[:, c, :], rhs, start=st, stop=sp)
                nc.vector.tensor_scalar_add(qt_sb[:, rsl], pqt, bq_col)
                nc.vector.tensor_scalar_add(kt_sb[:, rsl], pkt, bk_col)

            # ---- V: [keys, dims] with ones column (pad-masked) ----
            vaug = qkp.tile([128, 2, NKT, HD + 1], bf16, name=f"vaug{b}",
                            tag="vaug")
            for m in range(NKT):
                msl = slice(m * 128, (m + 1) * 128)
                pv = spool.tile([128, LD], f32, name="pv", tag="ps")
                for c in range(NCH):
                    nc.tensor.matmul(pv, xc[c][:, msl], wvT_sb[:, c, :],
                                     start=(c == 0), stop=(c == NCH - 1))
                tv = work.tile([128, LD], f32, name="tv", tag="tv")
                nc.vector.tensor_add(tv, pv, bv_bc)
                pcol = padf[:, b * NKT + m:b * NKT + m + 1]
                for h in range(2):
                    nc.vector.tensor_scalar_mul(
                        vaug[:, h, m, 0:HD], tv[:, h * HD:(h + 1) * HD], pcol)
                    nc.vector.tensor_copy(vaug[:, h, m, HD:HD + 1], pcol)

            # next batch's x^T load starts now (overlaps this batch's
            # attention; WAR on this batch's projection reads is tracked)
            if b + 1 < B:
                for rb in range(4):
                    rsl = slice(rb * 512, (rb + 1) * 512)
                    dsl = slice((b + 1) * S + rb * 512,
                                (b + 1) * S + (rb + 1) * 512)
                    for c in range(NCH):
                        nc.sync.dma_start(xc[c][:, rsl], xTr[c][:, dsl])

            # ---- attention in 512-query stripes ----
            stage = stg.tile([128, S], bf16, name=f"stage{b}", tag="stage")
            ests = [estp.tile([128, 2, blocklen], bf16, name=f"est{c}",
                              tag=f"est{c}")
                    for c, (_, blocklen) in enumerate(layout)]

            def do_scores(c, b=b, qt_sb=qt_sb, kt_sb=kt_sb, ests=ests):
                entries, _ = layout[c]
                est = ests[c]
                for kt, w, qs, off in entries:
                    ksl = slice(kt * 128, (kt + 1) * 128)
                    ps = spool.tile([128, 2, 512], f32, name="ps", tag="ps")
                    # both heads run concurrently in separate PE row groups
                    nc.tensor.matmul(ps[:, 0, 0:w], kt_sb[0:64, ksl],
                                     qt_sb[0:64, qs:qs + w],
                                     start=True, stop=True)
                    nc.tensor.matmul(ps[:, 1, 0:w], kt_sb[64:128, ksl],
                                     qt_sb[64:128, qs:qs + w],
                                     start=True, stop=True)
                    nc.scalar.activation(est[:, :, off:off + w], ps[:, :, 0:w],
                                         AF.Exp, scale=0.125)
                    if kt >= 4 * c:  # diagonal tile: causal mask
                        nc.vector.tensor_mul(est[:, :, off:off + 128],
                                             est[:, :, off:off + 128],
                                             diagmask2)

            pos = {}
            recbs = {}

            def do_pv(c, b=b, vaug=vaug, ests=ests, pos=pos, recbs=recbs):
                entries, _ = layout[c]
                est = ests[c]
                for h in range(2):
                    po = pvpool.tile([128, 512], f32, name=f"po{h}",
                                     tag=f"po{h}")
                    pos[(c, h)] = po
                    last = 4 * c + 3
                    for kt, w, qs, off in entries:
                        po_off = qs - 512 * c
                        nc.tensor.matmul(po[0:HD + 1, po_off:po_off + w],
                                         vaug[:, h, kt, :],
                                         est[:, h, off:off + w],
                                         start=(kt == 0), stop=(kt == last))
                    # start the reciprocal chain immediately (DVE) so the
                    # broadcast matmul issued later never stalls the PE
                    den = recp.tile([1, 512], f32, name="den", tag=f"den{h}")
                    # custom-DVE recip ignores the input base partition, so
                    # stage the denominator row at partition 0 first
                    nc.vector.tensor_copy(den, po[HD:HD + 1, :])
                    rec = recp.tile([1, 512], f32, name="rec", tag=f"rec{h}")
                    nc.vector.reciprocal_approx_fast(rec, den)
                    recb = recp.tile([1, 512], bf16, name="recb",
                                     tag=f"recb{h}", bufs=2)
                    nc.vector.tensor_copy(recb, rec)
                    recbs[(c, h)] = recb

            def do_div(c, stage=stage, pos=pos, recbs=recbs):
                # softmax division: broadcast 1/denominator to 64 partitions
                # via a tiny PE matmul into po's upper half, then multiply
                for h in range(2):
                    nc.tensor.matmul(pos[(c, h)][64:128, :], ones64,
                                     recbs[(c, h)],
                                     start=True, stop=True,
                                     skip_group_check=True)
                for h in range(2):
                    po = pos[(c, h)]
                    rbc = recp.tile([HD, 512], bf16, name="rbc", tag=f"rbc{h}")
                    nc.vector.tensor_copy(rbc, po[64:128, :])
                    nc.vector.tensor_mul(
                        stage[h * HD:(h + 1) * HD, 512 * c:512 * (c + 1)],
                        po[0:HD, :], rbc)

            def do_a2a(q0, q1, key, b=b, stage=stage):
                nq = (q1 - q0) // NCORES
                a2a_in = dpool.tile([NCORES * 128, nq], bf16,
                                    name=f"a2ai{key}", tag="a2ai", bufs=3)
                nc.gpsimd.dma_start(
                    a2a_in.rearrange("(j p) r -> p j r", p=128),
                    stage[:, q0:q1].rearrange("p (j r) -> p j r", j=NCORES))
                a2a_out = dpool.tile([NCORES * 128, nq], bf16,
                                     name=f"a2ao{key}", tag="a2ao", bufs=3)
                nc.gpsimd.collective_compute(
                    "AllToAll", ALU.bypass,
                    replica_groups=[list(range(NCORES))],
                    ins=[a2a_in.opt()], outs=[a2a_out.opt()])
                a2a_outs.append((b, q0, nq, a2a_out))

            # PV trails scores by one stripe; divisions trail by one more so
            # the broadcast matmuls never stall the PE FIFO
            do_scores(0)
            do_scores(1)
            do_pv(0)
            do_scores(2)
            do_pv(1)
            do_div(0)
            do_scores(3)
            do_pv(2)
            do_div(1)
            if b == 1:
                do_a2a(0, 1024, "b1t0")
            do_pv(3)
            do_div(2)
            do_div(3)
            if b == 0:
                do_a2a(0, S, "b0")
            else:
                do_a2a(1024, S, "b1t1")

        # ---- output projection (after both batches; overlaps tail A2A) ----
        for b, q0, nq, a2a_out in a2a_outs:
            a2a_sb = stg.tile([128, NCORES, nq], bf16, name=f"a2as{b}{q0}",
                              tag="a2as", bufs=3)
            nc.sync.dma_start(
                a2a_sb, a2a_out.rearrange("(j p) r -> p j r", p=128))
            for rc in range(nq // 128):
                r0 = b * RB + q0 // NCORES + rc * 128
                rsl = slice(rc * 128, (rc + 1) * 128)
                for n in range(D // 512):
                    pout = spool.tile([128, 512], f32, name="pout", tag="ps")
                    for c in range(NCH):
                        nc.tensor.matmul(
                            pout,
                            a2a_sb[:, c, rsl],
                            woT_sb[:, c, n * 512:(n + 1) * 512],
                            start=(c == 0), stop=(c == NCH - 1))
                    ot = work.tile([128, 512], f32, name="ot", tag="ot")
                    nc.vector.tensor_add(ot, pout,
                                         bo_bc[:, n * 512:(n + 1) * 512])
                    nc.sync.dma_start(
                        out.ap()[r0:r0 + 128, n * 512:(n + 1) * 512], ot)

    nc.finalize()
    return nc


def _get_nc():
    with _lock:
        if "nc" not in _cache:
            _cache["nc"] = _build_nc()
        return _cache["nc"]


def _shard_inputs(x, input_ids, Wq, bq, Wk, bk, Wv, bv, Wo, bo):
    import ml_dtypes
    bf16 = ml_dtypes.bfloat16

    x = np.asarray(x, dtype=np.float32)
    xT = np.ascontiguousarray(x.reshape(R, D).T).astype(bf16)
    woT = np.ascontiguousarray(np.asarray(Wo, dtype=np.float32).T).astype(bf16)
    bo_f = np.asarray(bo, dtype=np.float32)
    ids = np.asarray(input_ids).astype(np.int32)
    # ids_r[p, b*NKT + t] = input_ids[b, t*128 + p]
    ids_r = np.ascontiguousarray(ids.reshape(B, NKT, 128).transpose(2, 0, 1)
                                 .reshape(128, B * NKT))
    Wq = np.asarray(Wq, dtype=np.float32)
    Wk = np.asarray(Wk, dtype=np.float32)
    Wv = np.asarray(Wv, dtype=np.float32)
    bq = np.asarray(bq, dtype=np.float32)
    bk = np.asarray(bk, dtype=np.float32)
    bv = np.asarray(bv, dtype=np.float32)

    in_maps = []
    for c in range(NCORES):
        sl = slice(c * LD, (c + 1) * LD)
        in_maps.append({
            "xT": xT,
            "wqT": np.ascontiguousarray(Wq[sl].T).astype(bf16),
            "wkT": np.ascontiguousarray(Wk[sl].T).astype(bf16),
            "wvT": np.ascontiguousarray(Wv[sl].T).astype(bf16),
            "woT": woT,
            "bq": bq[sl].copy(),
            "bk": bk[sl].copy(),
            "bv": bv[sl].copy(),
            "bo": bo_f,
            "ids": ids_r,
        })
    return in_maps


def run(trace=False, **inputs):
    """Run the kernel; returns (output, BassKernelResults)."""
    from concourse.bass_utils import run_bass_kernel_spmd

    nc = _get_nc()
    in_maps = _shard_inputs(**inputs)
    res = run_bass_kernel_spmd(nc, in_maps, core_ids=list(range(NCORES)),
                               trace=trace)
    full = np.empty((B, S, D), dtype=np.float32)
    for c in range(NCORES):
        o = np.asarray(res.results[c]["out"], dtype=np.float32)
        # batch 0: one A2A, contiguous 256 queries per core
        full[0, c * RB:(c + 1) * RB, :] = o[0:RB, :]
        # batch 1: two A2A halves, 128 queries per core each
        for t in range(2):
            full[1, t * 1024 + c * 128:t * 1024 + (c + 1) * 128, :] = \
                o[RB + t * 128:RB + (t + 1) * 128, :]
    return full, res


def kernel(**inputs) -> np.ndarray:
    full, _ = run(trace=False, **inputs)
    return full


# revision 37
# speedup vs baseline: 1.1678x; 1.1678x over previous
"""Trainium2 Bass kernel for a causal multi-head attention layer.

Model: b=2, s=2048, d_model=1024, 16 heads, head_dim=64, pad-index 0.
Sharding over 8 NeuronCores: each core owns 2 heads (128 of the 1024
attention dims) for both batches (head/tensor parallel).  After attention,
an AllToAll redistributes the per-head outputs so each core holds all 1024
attention dims for 1/8 of the sequence positions, where it runs the output
projection locally.  Output rows per core: 256 rows of each batch.

Attention is computed in 512-query stripes: per stripe, scores for both
heads run as concurrent row-group matmuls (h0 in PE rows 0-63, h1 in
64-127), one exp covers both heads, and the PV matmul keeps V stationary
(with a ones column for the softmax denominator) so the output lands
directly in [dims, queries] layout for the AllToAll.
"""

import threading

import numpy as np

B, S, D = 2, 2048, 1024
H, HD = 16, 64
NCORES = 8
LD = D // NCORES          # 128 local attention dims (2 heads)
R = B * S                 # 4096 flattened rows
RC = R // NCORES          # 512 output rows per core
RB = S // NCORES          # 256 rows per batch per core
NKT = S // 128            # 16 key tiles per batch
NCH = D // 128            # 8 contraction chunks of d_model
NST = S // 512            # 4 query stripes per batch

_cache = {}
_lock = threading.Lock()


def _stripe_layout():
    """Per stripe c: list of (kt, width, q_start, offset-in-block), block len."""
    layout = []
    for c in range(NST):
        entries = []
        off = 0
        for kt in range(4 * c + 4):
            qs = max(512 * c, kt * 128)
            w = 512 * (c + 1) - qs
            entries.append((kt, w, qs, off))
            off += w
        layout.append((entries, off))
    return layout


def _build_nc():
    import concourse.mybir as mybir
    import concourse.tile as tile
    from concourse import bacc
    from contextlib import ExitStack

    f32 = mybir.dt.float32
    bf16 = mybir.dt.bfloat16
    i32 = mybir.dt.int32
    AF = mybir.ActivationFunctionType
    ALU = mybir.AluOpType

    nc = bacc.Bacc(None, target_bir_lowering=False, num_devices=NCORES)

    xT = nc.declare_dram_parameter("xT", [D, R], bf16, isOutput=False)
    wqT = nc.declare_dram_parameter("wqT", [D, LD], bf16, isOutput=False)
    wkT = nc.declare_dram_parameter("wkT", [D, LD], bf16, isOutput=False)
    wvT = nc.declare_dram_parameter("wvT", [D, LD], bf16, isOutput=False)
    woT = nc.declare_dram_parameter("woT", [D, D], bf16, isOutput=False)
    bq = nc.declare_dram_parameter("bq", [LD], f32, isOutput=False)
    bk = nc.declare_dram_parameter("bk", [LD], f32, isOutput=False)
    bv = nc.declare_dram_parameter("bv", [LD], f32, isOutput=False)
    bo = nc.declare_dram_parameter("bo", [D], f32, isOutput=False)
    ids = nc.declare_dram_parameter("ids", [128, B * NKT], i32, isOutput=False)
    out = nc.declare_dram_parameter("out", [RC, D], f32, isOutput=True)

    layout = _stripe_layout()

    with ExitStack() as ctx:
        tc = ctx.enter_context(tile.TileContext(nc))
        const = ctx.enter_context(tc.tile_pool(name="const", bufs=1))
        xcp = ctx.enter_context(tc.tile_pool(name="xcp", bufs=1))
        qkp = ctx.enter_context(tc.tile_pool(name="qkp", bufs=2))
        estp = ctx.enter_context(tc.tile_pool(name="estp", bufs=1))
        stg = ctx.enter_context(tc.tile_pool(name="stg", bufs=2))
        work = ctx.enter_context(tc.tile_pool(name="work", bufs=2))
        recp = ctx.enter_context(tc.tile_pool(name="recp", bufs=1))
        spool = ctx.enter_context(tc.tile_pool(name="spool", bufs=2, space="PSUM"))
        pvpool = ctx.enter_context(tc.tile_pool(name="pvpool", bufs=2, space="PSUM"))
        dpool = ctx.enter_context(tc.tile_pool(name="dram", bufs=2, space="DRAM"))

        # ---- constants (small weights first so compute can start early) ----
        wqT_sb = const.tile([128, NCH, LD], bf16)
        nc.sync.dma_start(wqT_sb, wqT.ap().rearrange("(c p) d -> p c d", p=128))
        wkT_sb = const.tile([128, NCH, LD], bf16)
        nc.sync.dma_start(wkT_sb, wkT.ap().rearrange("(c p) d -> p c d", p=128))
        wvT_sb = const.tile([128, NCH, LD], bf16)
        nc.sync.dma_start(wvT_sb, wvT.ap().rearrange("(c p) d -> p c d", p=128))

        bq_col = const.tile([128, 1], f32)
        nc.sync.dma_start(bq_col, bq.ap().rearrange("(p o) -> p o", o=1))
        bk_col = const.tile([128, 1], f32)
        nc.sync.dma_start(bk_col, bk.ap().rearrange("(p o) -> p o", o=1))
        bv_bc = const.tile([128, LD], f32)
        nc.sync.dma_start(bv_bc, bv.ap().partition_broadcast(128))

        ids_sb = const.tile([128, B * NKT], i32)
        nc.sync.dma_start(ids_sb, ids.ap())
        ones64 = const.tile([1, 64], bf16)
        nc.vector.memset(ones64, 1.0)

        # x^T for batch 0, chunked by (row-block, dim-chunk) so projection
        # matmuls start as soon as the first row block lands
        xTr = xT.ap().rearrange("(c p) r -> c p r", p=128)
        xc = [xcp.tile([128, S], bf16, name=f"xc{c}", tag=f"xc{c}")
              for c in range(NCH)]
        for rb in range(4):
            rsl = slice(rb * 512, (rb + 1) * 512)
            for c in range(NCH):
                nc.sync.dma_start(xc[c][:, rsl], xTr[c][:, rsl])

        woT_sb = const.tile([128, NCH, D], bf16)
        nc.sync.dma_start(woT_sb, woT.ap().rearrange("(c p) n -> p c n", p=128))
        bo_bc = const.tile([128, D], f32)
        nc.sync.dma_start(bo_bc, bo.ap().partition_broadcast(128))

        padf = const.tile([128, B * NKT], f32)
        nc.vector.tensor_copy(padf, ids_sb)
        nc.vector.tensor_scalar_min(padf, padf, 1.0)

        # diagmask2[x, h, y] = 1 if y >= x else 0 (keys on partitions)
        diagmask = const.tile([128, 128], bf16)
        nc.gpsimd.memset(diagmask, 1.0)
        nc.gpsimd.affine_select(
            out=diagmask, in_=diagmask, compare_op=ALU.is_ge, fill=0.0,
            base=0, pattern=[[1, 128]], channel_multiplier=-1,
        )
        diagmask2 = const.tile([128, 2, 128], bf16)
        nc.vector.tensor_copy(diagmask2[:, 0, :], diagmask)
        nc.vector.tensor_copy(diagmask2[:, 1, :], diagmask)

        a2a_outs = []

        def batch_flow(b):
            # ---- Q/K projections: [dims, rows], both heads stacked ----
            qt_sb = qkp.tile([128, S], bf16, name=f"qt{b}", tag="qt")
            kt_sb = qkp.tile([128, S], bf16, name=f"kt{b}", tag="kt")
            for rb in range(4):
                rsl = slice(rb * 512, (rb + 1) * 512)
                pqt = spool.tile([128, 512], f32, name="pqt", tag="ps")
                pkt = spool.tile([128, 512], f32, name="pkt", tag="ps")
                for c in range(NCH):
                    st = c == 0
                    sp = c == NCH - 1
                    rhs = xc[c][:, rsl]
                    nc.tensor.matmul(pqt, wqT_sb[:, c, :], rhs, start=st, stop=sp)
                    nc.tensor.matmul(pkt, wkT_sb[:, c, :], rhs, start=st, stop=sp)
                nc.vector.tensor_scalar_add(qt_sb[:, rsl], pqt, bq_col)
                nc.vector.tensor_scalar_add(kt_sb[:, rsl], pkt, bk_col)

            # ---- V: [keys, dims] with ones column (pad-masked) ----
            vaug = qkp.tile([128, 2, NKT, HD + 1], bf16, name=f"vaug{b}",
                            tag="vaug")
            for m in range(NKT):
                msl = slice(m * 128, (m + 1) * 128)
                pv = spool.tile([128, LD], f32, name="pv", tag="ps")
                for c in range(NCH):
                    nc.tensor.matmul(pv, xc[c][:, msl], wvT_sb[:, c, :],
                                     start=(c == 0), stop=(c == NCH - 1))
                tv = work.tile([128, LD], f32, name="tv", tag="tv")
                nc.vector.tensor_add(tv, pv, bv_bc)
                pcol = padf[:, b * NKT + m:b * NKT + m + 1]
                for h in range(2):
                    nc.vector.tensor_scalar_mul(
                        vaug[:, h, m, 0:HD], tv[:, h * HD:(h + 1) * HD], pcol)
                    nc.vector.tensor_copy(vaug[:, h, m, HD:HD + 1], pcol)

            # next batch's x^T load starts now (overlaps this batch's
            # attention; WAR on this batch's projection reads is tracked)
            if b + 1 < B:
                for rb in range(4):
                    rsl = slice(rb * 512, (rb + 1) * 512)
                    dsl = slice((b + 1) * S + rb * 512,
                                (b + 1) * S + (rb + 1) * 512)
                    for c in range(NCH):
                        nc.sync.dma_start(xc[c][:, rsl], xTr[c][:, dsl])

            yield  # proj done

            # ---- attention in 512-query stripes ----
            stage = stg.tile([128, S], bf16, name=f"stage{b}", tag="stage")
            ests = [estp.tile([128, 2, blocklen], bf16, name=f"est{c}",
                              tag=f"est{c}")
                    for c, (_, blocklen) in enumerate(layout)]

            def do_scores(c, b=b, qt_sb=qt_sb, kt_sb=kt_sb, ests=ests):
                entries, _ = layout[c]
                est = ests[c]
                for kt, w, qs, off in entries:
                    ksl = slice(kt * 128, (kt + 1) * 128)
                    ps = spool.tile([128, 2, 512], f32, name="ps", tag="ps")
                    # both heads run concurrently in separate PE row groups
                    nc.tensor.matmul(ps[:, 0, 0:w], kt_sb[0:64, ksl],
                                     qt_sb[0:64, qs:qs + w],
                                     start=True, stop=True)
                    nc.tensor.matmul(ps[:, 1, 0:w], kt_sb[64:128, ksl],
                                     qt_sb[64:128, qs:qs + w],
                                     start=True, stop=True)
                    nc.scalar.activation(est[:, :, off:off + w], ps[:, :, 0:w],
                                         AF.Exp, scale=0.125)
                    if kt >= 4 * c:  # diagonal tile: causal mask
                        nc.vector.tensor_mul(est[:, :, off:off + 128],
                                             est[:, :, off:off + 128],
                                             diagmask2)

            pos = {}
            recbs = {}

            def do_pv(c, b=b, vaug=vaug, ests=ests, pos=pos, recbs=recbs):
                entries, _ = layout[c]
                est = ests[c]
                for h in range(2):
                    po = pvpool.tile([128, 512], f32, name=f"po{h}",
                                     tag=f"po{h}")
                    pos[(c, h)] = po
                    last = 4 * c + 3
                    for kt, w, qs, off in entries:
                        po_off = qs - 512 * c
                        nc.tensor.matmul(po[0:HD + 1, po_off:po_off + w],
                                         vaug[:, h, kt, :],
                                         est[:, h, off:off + w],
                                         start=(kt == 0), stop=(kt == last))
                    # start the reciprocal chain immediately (DVE) so the
                    # broadcast matmul issued later never stalls the PE
                    den = recp.tile([1, 512], f32, name="den", tag=f"den{h}")
                    # custom-DVE recip ignores the input base partition, so
                    # stage the denominator row at partition 0 first
                    nc.vector.tensor_copy(den, po[HD:HD + 1, :])
                    rec = recp.tile([1, 512], f32, name="rec", tag=f"rec{h}")
                    nc.vector.reciprocal_approx_fast(rec, den)
                    recb = recp.tile([1, 512], bf16, name="recb",
                                     tag=f"recb{h}", bufs=2)
                    nc.vector.tensor_copy(recb, rec)
                    recbs[(c, h)] = recb

            def do_div(c, stage=stage, pos=pos, recbs=recbs):
                # softmax division: broadcast 1/denominator to 64 partitions
                # via a tiny PE matmul into po's upper half, then multiply
                for h in range(2):
                    nc.tensor.matmul(pos[(c, h)][64:128, :], ones64,
                                     recbs[(c, h)],
                                     start=True, stop=True,
                                     skip_group_check=True)
                for h in range(2):
                    po = pos[(c, h)]
                    rbc = recp.tile([HD, 512], bf16, name="rbc", tag=f"rbc{h}")
                    nc.vector.tensor_copy(rbc, po[64:128, :])
                    nc.vector.tensor_mul(
                        stage[h * HD:(h + 1) * HD, 512 * c:512 * (c + 1)],
                        po[0:HD, :], rbc)

            def do_a2a(q0, q1, key, b=b, stage=stage):
                nq = (q1 - q0) // NCORES
                a2a_in = dpool.tile([NCORES * 128, nq], bf16,
                                    name=f"a2ai{key}", tag="a2ai", bufs=3)
                nc.gpsimd.dma_start(
                    a2a_in.rearrange("(j p) r -> p j r", p=128),
                    stage[:, q0:q1].rearrange("p (j r) -> p j r", j=NCORES))
                a2a_out = dpool.tile([NCORES * 128, nq], bf16,
                                     name=f"a2ao{key}", tag="a2ao", bufs=3)
                nc.gpsimd.collective_compute(
                    "AllToAll", ALU.bypass,
                    replica_groups=[list(range(NCORES))],
                    ins=[a2a_in.opt()], outs=[a2a_out.opt()])
                a2a_outs.append((b, q0, nq, a2a_out))

            # PV trails scores by one stripe; divisions trail by one more so
            # the broadcast matmuls never stall the PE FIFO
            do_scores(0)
            do_scores(1)
            do_pv(0)
            do_scores(2)
            do_pv(1)
            do_div(0)
            do_scores(3)
            yield  # scores done — next batch's projections slot in here
            #       (fills PE while stripe-2/3 exps drain on the ACT engine)
            do_pv(2)
            do_div(1)
            if b == 1:
                do_a2a(0, 1024, "b1t0")
            do_pv(3)
            do_div(2)
            do_div(3)
            if b == 0:
                do_a2a(0, S, "b0")
            else:
                do_a2a(1024, S, "b1t1")

        # interleave: b1's projections are emitted between b0's scores and
        # b0's tail so the PE fills b0's exp-lag with projection work
        g0 = batch_flow(0)
        g1 = batch_flow(1)
        next(g0)            # b0 projections
        next(g0)            # b0 scores (stripes 0-3) + PV 0-1
        next(g1)            # b1 projections
        for _ in g0:        # b0 tail: PV 2-3, divisions, A2A
            pass
        for _ in g1:        # b1 scores + tail
            pass

        # ---- output projection (after both batches; overlaps tail A2A) ----
        for b, q0, nq, a2a_out in a2a_outs:
            a2a_sb = stg.tile([128, NCORES, nq], bf16, name=f"a2as{b}{q0}",
                              tag="a2as", bufs=3)
            nc.sync.dma_start(
                a2a_sb, a2a_out.rearrange("(j p) r -> p j r", p=128))
            for rc in range(nq // 128):
                r0 = b * RB + q0 // NCORES + rc * 128
                rsl = slice(rc * 128, (rc + 1) * 128)
                for n in range(D // 512):
                    pout = spool.tile([128, 512], f32, name="pout", tag="ps")
                    for c in range(NCH):
                        nc.tensor.matmul(
                            pout,
                            a2a_sb[:, c, rsl],
                            woT_sb[:, c, n * 512:(n + 1) * 512],
                            start=(c == 0), stop=(c == NCH - 1))
                    ot = work.tile([128, 512], f32, name="ot", tag="ot")
                    nc.vector.tensor_add(ot, pout,
                                         bo_bc[:, n * 512:(n + 1) * 512])
                    nc.sync.dma_start(
                        out.ap()[r0:r0 + 128, n * 512:(n + 1) * 512], ot)

    nc.finalize()
    return nc


def _get_nc():
    with _lock:
        if "nc" not in _cache:
            _cache["nc"] = _build_nc()
        return _cache["nc"]


def _shard_inputs(x, input_ids, Wq, bq, Wk, bk, Wv, bv, Wo, bo):
    import ml_dtypes
    bf16 = ml_dtypes.bfloat16

    x = np.asarray(x, dtype=np.float32)
    xT = np.ascontiguousarray(x.reshape(R, D).T).astype(bf16)
    woT = np.ascontiguousarray(np.asarray(Wo, dtype=np.float32).T).astype(bf16)
    bo_f = np.asarray(bo, dtype=np.float32)
    ids = np.asarray(input_ids).astype(np.int32)
    # ids_r[p, b*NKT + t] = input_ids[b, t*128 + p]
    ids_r = np.ascontiguousarray(ids.reshape(B, NKT, 128).transpose(2, 0, 1)
                                 .reshape(128, B * NKT))
    Wq = np.asarray(Wq, dtype=np.float32)
    Wk = np.asarray(Wk, dtype=np.float32)
    Wv = np.asarray(Wv, dtype=np.float32)
    bq = np.asarray(bq, dtype=np.float32)
    bk = np.asarray(bk, dtype=np.float32)
    bv = np.asarray(bv, dtype=np.float32)

    in_maps = []
    for c in range(NCORES):
        sl = slice(c * LD, (c + 1) * LD)
        in_maps.append({
            "xT": xT,
            "wqT": np.ascontiguousarray(Wq[sl].T).astype(bf16),
            "wkT": np.ascontiguousarray(Wk[sl].T).astype(bf16),
            "wvT": np.ascontiguousarray(Wv[sl].T).astype(bf16),
            "woT": woT,
            "bq": bq[sl].copy(),
            "bk": bk[sl].copy(),
            "bv": bv[sl].copy(),
            "bo": bo_f,
            "ids": ids_r,
        })
    return in_maps


def run(trace=False, **inputs):
    """Run the kernel; returns (output, BassKernelResults)."""
    from concourse.bass_utils import run_bass_kernel_spmd

    nc = _get_nc()
    in_maps = _shard_inputs(**inputs)
    res = run_bass_kernel_spmd(nc, in_maps, core_ids=list(range(NCORES)),
                               trace=trace)
    full = np.empty((B, S, D), dtype=np.float32)
    for c in range(NCORES):
        o = np.asarray(res.results[c]["out"], dtype=np.float32)
        # batch 0: one A2A, contiguous 256 queries per core
        full[0, c * RB:(c + 1) * RB, :] = o[0:RB, :]
        # batch 1: two A2A halves, 128 queries per core each
        for t in range(2):
            full[1, t * 1024 + c * 128:t * 1024 + (c + 1) * 128, :] = \
                o[RB + t * 128:RB + (t + 1) * 128, :]
    return full, res


def kernel(**inputs) -> np.ndarray:
    full, _ = run(trace=False, **inputs)
    return full
